# revision 18
# baseline (speedup 1.0000x reference)
"""DistanceAttentionPerPosition Trainium2 kernel (8-core data parallel).

Math restructure vs the reference:
  hidden = gelu([e1; e2; d*mask; 1] @ [w_in; b_in])   (embeddings gathered on
  host into a [66, edges] bf16 operand, d-major edge order; contraction 66)
  scores = gelu(hidden@w_a1)@w_a2  (b_a2 cancels in softmax)
  out = (sum_d attn_d * gelu(hidden@w_v1 + b_v1)) @ w_v2 + b_v2   (sum attn = 1)
Per core: 512 positions x 64 edges; 4 groups of 128 positions.

All matmul operands are bf16 (f32 PSUM accumulate). The bias matmuls are
skipped when the host sees all-zero b_v1/b_a1/b_v2 (build-time constant
folding; nonzero biases take the general path). Per group, phase 2 of the
NEXT group is emitted between softmax and the weighted sum so the PE/ACT
engines stay busy while the DVE drains the attention reduction; PSUM is one
unified 2x[128,4,512] pool so gelu ops cover 4 slots each.

Built on bacc.Bacc (its generate_event_semaphores pass splits multi-waits that
this walrus rejects).
"""

import sys
import numpy as np

sys.path.insert(0, "/opt/trn_rl_repo")

from contextlib import ExitStack

import concourse.bass as bass
import concourse.bacc as bacc
import concourse.tile as tile
from concourse import mybir
from concourse.bass_utils import run_bass_kernel_spmd

F32 = mybir.dt.float32
AX = mybir.AxisListType
ALU = mybir.AluOpType
ACTF = mybir.ActivationFunctionType

B, W, D = 16, 256, 64
E, H = 32, 256
NT = 101
NCORES = 8
PC = (B * W) // NCORES      # positions per core = 512
NE = PC * D                 # edges per core = 32768
G = PC // 128               # groups per core = 4
CHUNK = 512
NCHUNK = NE // CHUNK        # 64
NJ = D // 2
CIN = 2 * E + 2             # phase-1 contraction rows: e1, e2, d*mask, ones

# bf16 const pack (matmul operands), one [128, CR] tensor
O_WC = 0            # wcat  [128, 2*320]
O_WV = 640          # wv2   [128, 2*256]
O_ON = 1152         # ones  [row0, 128]
O_BC = 1280         # bcat  [row0, 320]
O_BV = 1600         # bv2   [row0, 256]
CR = 1856
# f32 const pack, one [128, CF] tensor
O_ID = 0            # ident [128, 128]
O_WA = 128          # wa2b  [128, 2*64]
CF = 256

TRACE = False
LAST_EXEC_NS = None


def build_nc(gelu=None, zero_bcat=False, zero_bv2=False):
    gelu = ACTF.Gelu if gelu is None else gelu
    ACTF_Gelu = gelu
    nc = bacc.Bacc(None, target_bir_lowering=False)

    F32R = mybir.dt.float32r
    BF16 = mybir.dt.bfloat16
    combD = nc.declare_dram_parameter("comb", [G, CIN, 128 * D], BF16, isOutput=False)
    winD = nc.declare_dram_parameter("win", [CIN, H], BF16, isOutput=False)
    sbD = nc.declare_dram_parameter("sbias", [G, 128, D], F32, isOutput=False)
    cD = nc.declare_dram_parameter("constsr", [128, CR], BF16, isOutput=False)
    cfD = nc.declare_dram_parameter("constsf", [128, CF], F32, isOutput=False)
    outD = nc.declare_dram_parameter("out", [PC, H], F32, isOutput=True)

    with tile.TileContext(nc) as tc, ExitStack() as ctx:
        const = ctx.enter_context(tc.tile_pool(name="const", bufs=1))
        cbp = ctx.enter_context(tc.tile_pool(name="cbp", bufs=2))
        gpp = ctx.enter_context(tc.tile_pool(name="gpp", bufs=2))
        gvp = ctx.enter_context(tc.tile_pool(name="gvp", bufs=2))
        scp = ctx.enter_context(tc.tile_pool(name="scp", bufs=2))
        vp = ctx.enter_context(tc.tile_pool(name="vp", bufs=2))
        outp = ctx.enter_context(tc.tile_pool(name="outp", bufs=2))
        scsp = ctx.enter_context(tc.tile_pool(name="scsp", bufs=1))
        ps8 = ctx.enter_context(
            tc.tile_pool(name="ps8", bufs=2, space=bass.MemorySpace.PSUM))

        C = const.tile([128, CR], BF16, tag="constsr")
        nc.sync.dma_start(C[:], cD[:])
        Cf = const.tile([128, CF], F32, tag="constsf")
        nc.sync.dma_start(Cf[:], cfD[:])
        Wb = const.tile([CIN, H], BF16, tag="win")
        nc.sync.dma_start(Wb[:], winD[:])
        def r(ap):
            return ap
        ones1 = C[0:1, O_ON:O_ON + 128]
        idn = Cf[:, O_ID:O_ID + 128]

        def phase1(g):
            gp = gpp.tile([128, 2, 128 * D], BF16, tag="gp")
            cb = cbp.tile([CIN, 128 * D], BF16, tag="cb")
            nc.sync.dma_start(cb[:], combD[g])
            for cp in range(NCHUNK // G // 2):
                pp = ps8.tile([128, 4, CHUNK], F32, tag="ps")
                for cc in range(2):
                    c = 2 * cp + cc
                    for m in range(2):
                        nc.tensor.matmul(pp[:, 2 * cc + m, :],
                                         Wb[:, m * 128:(m + 1) * 128],
                                         cb[:, c * CHUNK:(c + 1) * CHUNK],
                                         start=True, stop=True)
                nc.scalar.activation(
                    gp[:, :, cp * 2 * CHUNK:(cp + 1) * 2 * CHUNK].rearrange(
                        "p m (cc e) -> p cc m e", cc=2),
                    pp[:, :, :], ACTF_Gelu)
            return gp

        def phase1_tile(state, cp):
            gp, cb = state
            pp = ps8.tile([128, 4, CHUNK], F32, tag="ps")
            for cc in range(2):
                c = 2 * cp + cc
                for m in range(2):
                    nc.tensor.matmul(pp[:, 2 * cc + m, :],
                                     Wb[:, m * 128:(m + 1) * 128],
                                     cb[:, c * CHUNK:(c + 1) * CHUNK],
                                     start=True, stop=True)
            nc.scalar.activation(
                gp[:, :, cp * 2 * CHUNK:(cp + 1) * 2 * CHUNK].rearrange(
                    "p m (cc e) -> p cc m e", cc=2),
                pp[:, :, :], ACTF_Gelu)

        def phase2_half(gp, gva, half):
            for j in range(half * (D // 8), (half + 1) * (D // 8)):
                vps = ps8.tile([128, 4, CHUNK], F32, tag="ps")
                for dd in range(4):
                    d = 4 * j + dd
                    if not zero_bcat:
                        nc.tensor.matmul(vps[:, dd, 0:320], r(ones1),
                                         r(C[0:1, O_BC:O_BC + 320]),
                                         start=True, stop=False)
                    for k in range(2):
                        nc.tensor.matmul(
                            vps[:, dd, 0:320],
                            r(gp[:, k, d * 128:(d + 1) * 128]),
                            r(C[:, O_WC + k * 320:O_WC + (k + 1) * 320]),
                            start=(zero_bcat and k == 0), stop=(k == 1))
                nc.scalar.activation(gva[:, 4 * j:4 * j + 4, :], vps[:, 0:4, 0:320],
                                     ACTF_Gelu)

        def phase12(g):
            # phase 1 emitted in two halves around phase 2's first half:
            # with d-major ordering p2 tiles j<8 only need p1 chunks <8, so
            # ACT gets value-gelu work after half the phase-1 matmul burst
            gp = gpp.tile([128, 2, 128 * D], BF16, tag="gp")
            cb = cbp.tile([CIN, 128 * D], BF16, tag="cb")
            nc.sync.dma_start(cb[:], combD[g])
            gva = gvp.tile([128, D, 320], BF16, tag="gva")
            for cp in range(4):
                phase1_tile((gp, cb), cp)
            phase2_half(gp, gva, 0)
            for cp in range(4, 8):
                phase1_tile((gp, cb), cp)
            phase2_half(gp, gva, 1)
            return gva

        gva = phase12(0)
        for g in range(G):
            # ---- phase 3: scores + softmax over d (DVE/ACT) ----
            sc = scp.tile([128, D], F32, tag="sc")
            scs = scsp.tile([128, D, 64], F32, tag="scs")
            DS = 48
            nc.vector.tensor_tensor(
                scs[:, 0:DS, :], gva[:, 0:DS, 256:320],
                Cf[:, O_WA:O_WA + 64][:, None, :].broadcast_to([128, DS, 64]),
                ALU.mult)
            nc.vector.tensor_reduce(sc[:, 0:DS], scs[:, 0:DS, :], AX.X, ALU.add)
            nc.vector.tensor_tensor(
                scs[:, DS:D, :], gva[:, DS:D, 256:320],
                Cf[:, O_WA:O_WA + 64][:, None, :].broadcast_to([128, D - DS, 64]),
                ALU.mult)
            nc.vector.tensor_reduce(sc[:, DS:D], scs[:, DS:D, :], AX.X, ALU.add)
            sb = scp.tile([128, D], F32, tag="sb")
            nc.gpsimd.dma_start(sb[:], sbD[g])
            nc.vector.tensor_tensor(sc[:], sc[:], sb[:], ALU.add)
            at = scp.tile([128, D], F32, tag="at")
            sm = scp.tile([128, 1], F32, tag="sm")
            nc.scalar.activation(at[:], sc[:], ACTF.Exp, accum_out=sm[:])
            rc = scp.tile([128, 1], F32, tag="rc")
            nc.vector.reciprocal(rc[:], sm[:])
            nc.vector.tensor_scalar(at[:], at[:], rc[:], None, ALU.mult)

            # next group's phase 2 keeps PE/ACT busy while DVE does
            # this group's softmax + weighted sum
            gva_next = None
            if g + 1 < G:
                gva_next = phase12(g + 1)

            # ---- phase 4: V = sum_d attn_d * gv_d (DVE) ----
            V = vp.tile([128, H], F32, tag="V")
            nc.vector.tensor_scalar(V[:], gva[:, 0, 0:H], at[:, 0:1], None, ALU.mult)
            for d in range(1, D):
                nc.vector.scalar_tensor_tensor(
                    V[:], gva[:, d, 0:H], at[:, d:d + 1], V[:], ALU.mult, ALU.add)

            # ---- phase 5: out = V @ w_v2 + b_v2 ----
            vt_ps = ps8.tile([128, 4, CHUNK], F32, tag="ps")
            for k in range(2):
                nc.tensor.transpose(vt_ps[:, k, 0:128], V[:, bass.ts(k, 128)], idn)
            vt = vp.tile([128, 2, 128], BF16, tag="vt")
            for k in range(2):
                nc.vector.tensor_copy(vt[:, k, :], vt_ps[:, k, 0:128])
            fo = ps8.tile([128, 4, CHUNK], F32, tag="ps")
            if not zero_bv2:
                nc.tensor.matmul(fo[:, 0, 0:H], r(ones1), r(C[0:1, O_BV:O_BV + H]),
                                 start=True, stop=False)
            for k in range(2):
                nc.tensor.matmul(fo[:, 0, 0:H], r(vt[:, k, :]),
                                 r(C[:, O_WV + k * H:O_WV + (k + 1) * H]),
                                 start=(zero_bv2 and k == 0), stop=(k == 1))
            ot = outp.tile([128, H], F32, tag="ot")
            nc.scalar.copy(ot[:], fo[:, 0, 0:H])
            nc.sync.dma_start(outD[bass.ts(g, 128)], ot[:])
            gva = gva_next

    nc.compile()
    return nc


def _prep(inputs):
    import ml_dtypes
    BF = ml_dtypes.bfloat16

    a1 = np.asarray(inputs["atom1_idx"]).reshape(B * W, D)
    a2 = np.asarray(inputs["atom2_idx"]).reshape(B * W, D)
    dist = np.asarray(inputs["distances"], dtype=np.float32).reshape(B * W, D)
    mask = np.asarray(inputs["mask"]).astype(np.float32).reshape(B * W, D)
    dm = dist * mask
    sbias = (mask - 1.0) * 1e4

    ae = np.asarray(inputs["atom_embed"], dtype=np.float32).copy()
    ae[NT - 1] = 0.0
    w_in = np.asarray(inputs["w_in"], dtype=np.float32)

    win = np.zeros((CIN, H), np.float32)
    win[0:2 * E] = w_in[0:2 * E]
    win[2 * E] = w_in[2 * E]
    win[2 * E + 1] = np.asarray(inputs["b_in"], dtype=np.float32)
    win16 = win.astype(BF)

    consts = np.zeros((128, CR), np.float32)
    w_v1 = np.asarray(inputs["w_v1"], dtype=np.float32)
    w_a1 = np.asarray(inputs["w_a1"], dtype=np.float32)
    wcat = np.concatenate([w_v1, w_a1], axis=1)          # [256, 320]
    consts[:, O_WC:O_WC + 320] = wcat[0:128]
    consts[:, O_WC + 320:O_WC + 640] = wcat[128:256]
    wv2 = np.asarray(inputs["w_v2"], dtype=np.float32)
    consts[:, O_WV:O_WV + H] = wv2[0:128]
    consts[:, O_WV + H:O_WV + 2 * H] = wv2[128:256]
    consts[0, O_ON:O_ON + 128] = 1.0
    consts[0, O_BC:O_BC + 320] = np.concatenate(
        [np.asarray(inputs["b_v1"], dtype=np.float32),
         np.asarray(inputs["b_a1"], dtype=np.float32)])
    consts[0, O_BV:O_BV + H] = np.asarray(inputs["b_v2"], dtype=np.float32)
    constsf = np.zeros((128, CF), np.float32)
    constsf[:, O_ID:O_ID + 128] = np.eye(128, dtype=np.float32)
    wa2 = np.asarray(inputs["w_a2"], dtype=np.float32)[:, 0]
    constsf[:, O_WA:O_WA + 128] = np.tile(wa2, 2)[None, :]

    e1 = ae[a1]                        # [B*W, D, E]
    e2 = ae[a2]

    maps = []
    for c in range(NCORES):
        s = slice(c * PC, (c + 1) * PC)
        m = dict(constsr=consts.astype(BF), constsf=constsf, win=win16)
        comb = np.empty((G, CIN, 128 * D), np.float32)
        comb[:, 0:E] = e1[s].reshape(G, 128, D, E).transpose(0, 3, 2, 1).reshape(
            G, E, 128 * D)
        comb[:, E:2 * E] = e2[s].reshape(G, 128, D, E).transpose(0, 3, 2, 1).reshape(
            G, E, 128 * D)
        comb[:, 2 * E] = dm[s].reshape(G, 128, D).transpose(0, 2, 1).reshape(
            G, 128 * D)
        comb[:, 2 * E + 1] = 1.0
        m["comb"] = comb.astype(BF)
        m["sbias"] = sbias[s].reshape(G, 128, D).astype(np.float32)
        maps.append(m)
    return maps, mask


def kernel(**inputs):
    global LAST_EXEC_NS
    maps, mask = _prep(inputs)
    zb1 = (not np.any(np.asarray(inputs["b_v1"]))) and (
        not np.any(np.asarray(inputs["b_a1"])))
    zb2 = not np.any(np.asarray(inputs["b_v2"]))
    nc = build_nc(None, zero_bcat=zb1, zero_bv2=zb2)
    res = run_bass_kernel_spmd(nc, maps, list(range(NCORES)), trace=TRACE)
    LAST_EXEC_NS = res.exec_time_ns
    out = np.concatenate([res.results[c]["out"] for c in range(NCORES)], axis=0)
    out = out.reshape(B, W, H)
    any_valid = mask.reshape(B, W, D).any(axis=2)
    fb = np.asarray(inputs["fallback"], dtype=np.float32)
    out = np.where(any_valid[..., None], out, fb[None, None, :])
    return out.astype(np.float32)


if __name__ == "__main__":
    nc = build_nc()
    print("build ok")


# revision 19
# speedup vs baseline: 1.0099x; 1.0099x over previous
"""DistanceAttentionPerPosition Trainium2 kernel (8-core data parallel).

Math restructure vs the reference:
  hidden = gelu([e1; e2; d*mask; 1] @ [w_in; b_in])   (embeddings gathered on
  host into a [66, edges] bf16 operand, d-major edge order; contraction 66)
  scores = gelu(hidden@w_a1)@w_a2  (b_a2 cancels in softmax)
  out = (sum_d attn_d * gelu(hidden@w_v1 + b_v1)) @ w_v2 + b_v2   (sum attn = 1)
Per core: 512 positions x 64 edges; 4 groups of 128 positions.

All matmul operands are bf16 (f32 PSUM accumulate). The bias matmuls are
skipped when the host sees all-zero b_v1/b_a1/b_v2 (build-time constant
folding; nonzero biases take the general path). Per group, phase 2 of the
NEXT group is emitted between softmax and the weighted sum so the PE/ACT
engines stay busy while the DVE drains the attention reduction; PSUM is one
unified 2x[128,4,512] pool so gelu ops cover 4 slots each.

Built on bacc.Bacc (its generate_event_semaphores pass splits multi-waits that
this walrus rejects).
"""

import sys
import numpy as np

sys.path.insert(0, "/opt/trn_rl_repo")

from contextlib import ExitStack

import concourse.bass as bass
import concourse.bacc as bacc
import concourse.tile as tile
from concourse import mybir
from concourse.bass_utils import run_bass_kernel_spmd

F32 = mybir.dt.float32
AX = mybir.AxisListType
ALU = mybir.AluOpType
ACTF = mybir.ActivationFunctionType

B, W, D = 16, 256, 64
E, H = 32, 256
NT = 101
NCORES = 8
PC = (B * W) // NCORES      # positions per core = 512
NE = PC * D                 # edges per core = 32768
G = PC // 128               # groups per core = 4
CHUNK = 512
NCHUNK = NE // CHUNK        # 64
NJ = D // 2
CIN = 2 * E + 2             # phase-1 contraction rows: e1, e2, d*mask, ones

# bf16 const pack (matmul operands), one [128, CR] tensor
O_WC = 0            # wcat  [128, 2*320]
O_WV = 640          # wv2   [128, 2*256]
O_ON = 1152         # ones  [row0, 128]
O_BC = 1280         # bcat  [row0, 320]
O_BV = 1600         # bv2   [row0, 256]
CR = 1856
# f32 const pack, one [128, CF] tensor
O_ID = 0            # ident [128, 128]
O_WA = 128          # wa2b  [128, 2*64]
CF = 256

TRACE = False
LAST_EXEC_NS = None


def build_nc(gelu=None, zero_bcat=False, zero_bv2=False):
    gelu = ACTF.Gelu if gelu is None else gelu
    ACTF_Gelu = gelu
    nc = bacc.Bacc(None, target_bir_lowering=False)

    F32R = mybir.dt.float32r
    BF16 = mybir.dt.bfloat16
    combD = nc.declare_dram_parameter("comb", [G, CIN, 128 * D], BF16, isOutput=False)
    winD = nc.declare_dram_parameter("win", [CIN, H], BF16, isOutput=False)
    sbD = nc.declare_dram_parameter("sbias", [G, 128, D], F32, isOutput=False)
    cD = nc.declare_dram_parameter("constsr", [128, CR], BF16, isOutput=False)
    cfD = nc.declare_dram_parameter("constsf", [128, CF], F32, isOutput=False)
    outD = nc.declare_dram_parameter("out", [PC, H], F32, isOutput=True)

    with tile.TileContext(nc) as tc, ExitStack() as ctx:
        const = ctx.enter_context(tc.tile_pool(name="const", bufs=1))
        cbp = ctx.enter_context(tc.tile_pool(name="cbp", bufs=2))
        gpp = ctx.enter_context(tc.tile_pool(name="gpp", bufs=2))
        gvp = ctx.enter_context(tc.tile_pool(name="gvp", bufs=2))
        scp = ctx.enter_context(tc.tile_pool(name="scp", bufs=2))
        vp = ctx.enter_context(tc.tile_pool(name="vp", bufs=2))
        outp = ctx.enter_context(tc.tile_pool(name="outp", bufs=2))
        scsp = ctx.enter_context(tc.tile_pool(name="scsp", bufs=1))
        ps8 = ctx.enter_context(
            tc.tile_pool(name="ps8", bufs=2, space=bass.MemorySpace.PSUM))

        C = const.tile([128, CR], BF16, tag="constsr")
        nc.sync.dma_start(C[:], cD[:])
        Cf = const.tile([128, CF], F32, tag="constsf")
        nc.sync.dma_start(Cf[:], cfD[:])
        Wb = const.tile([CIN, H], BF16, tag="win")
        nc.sync.dma_start(Wb[:], winD[:])
        def r(ap):
            return ap
        ones1 = C[0:1, O_ON:O_ON + 128]
        idn = Cf[:, O_ID:O_ID + 128]

        def phase1(g):
            gp = gpp.tile([128, 2, 128 * D], BF16, tag="gp")
            cb = cbp.tile([CIN, 128 * D], BF16, tag="cb")
            nc.sync.dma_start(cb[:], combD[g])
            for cp in range(NCHUNK // G // 2):
                pp = ps8.tile([128, 4, CHUNK], F32, tag="ps")
                for cc in range(2):
                    c = 2 * cp + cc
                    for m in range(2):
                        nc.tensor.matmul(pp[:, 2 * cc + m, :],
                                         Wb[:, m * 128:(m + 1) * 128],
                                         cb[:, c * CHUNK:(c + 1) * CHUNK],
                                         start=True, stop=True)
                nc.scalar.activation(
                    gp[:, :, cp * 2 * CHUNK:(cp + 1) * 2 * CHUNK].rearrange(
                        "p m (cc e) -> p cc m e", cc=2),
                    pp[:, :, :], ACTF_Gelu)
            return gp

        def phase1_tile(state, cp):
            gp, cb = state
            pp = ps8.tile([128, 4, CHUNK], F32, tag="ps")
            for cc in range(2):
                c = 2 * cp + cc
                for m in range(2):
                    nc.tensor.matmul(pp[:, 2 * cc + m, :],
                                     Wb[:, m * 128:(m + 1) * 128],
                                     cb[:, c * CHUNK:(c + 1) * CHUNK],
                                     start=True, stop=True)
            nc.scalar.activation(
                gp[:, :, cp * 2 * CHUNK:(cp + 1) * 2 * CHUNK].rearrange(
                    "p m (cc e) -> p cc m e", cc=2),
                pp[:, :, :], ACTF_Gelu)

        def phase2(gp):
            # values + attention-logit inputs: gva = gelu(hidden @ wcat [+ bcat])
            gva = gvp.tile([128, D, 320], BF16, tag="gva")
            for j in range(D // 4):
                vps = ps8.tile([128, 4, CHUNK], F32, tag="ps")
                for dd in range(4):
                    d = 4 * j + dd
                    if not zero_bcat:
                        nc.tensor.matmul(vps[:, dd, 0:320], r(ones1),
                                         r(C[0:1, O_BC:O_BC + 320]),
                                         start=True, stop=False)
                    for k in range(2):
                        nc.tensor.matmul(
                            vps[:, dd, 0:320],
                            r(gp[:, k, d * 128:(d + 1) * 128]),
                            r(C[:, O_WC + k * 320:O_WC + (k + 1) * 320]),
                            start=(zero_bcat and k == 0), stop=(k == 1))
                nc.scalar.activation(gva[:, 4 * j:4 * j + 4, :], vps[:, 0:4, 0:320],
                                     ACTF_Gelu)
            return gva

        def phase1(g):
            gp = gpp.tile([128, 2, 128 * D], BF16, tag="gp")
            cb = cbp.tile([CIN, 128 * D], BF16, tag="cb")
            nc.sync.dma_start(cb[:], combD[g])
            for cp in range(NCHUNK // G // 2):
                phase1_tile((gp, cb), cp)
            return gp

        gva = phase2(phase1(0))
        for g in range(G):
            # ---- phase 3: scores + softmax over d (DVE/ACT) ----
            sc = scp.tile([128, D], F32, tag="sc")
            scs = scsp.tile([128, D, 64], F32, tag="scs")
            DS = 48
            nc.vector.tensor_tensor(
                scs[:, 0:DS, :], gva[:, 0:DS, 256:320],
                Cf[:, O_WA:O_WA + 64][:, None, :].broadcast_to([128, DS, 64]),
                ALU.mult)
            nc.vector.tensor_reduce(sc[:, 0:DS], scs[:, 0:DS, :], AX.X, ALU.add)
            nc.vector.tensor_tensor(
                scs[:, DS:D, :], gva[:, DS:D, 256:320],
                Cf[:, O_WA:O_WA + 64][:, None, :].broadcast_to([128, D - DS, 64]),
                ALU.mult)
            nc.vector.tensor_reduce(sc[:, DS:D], scs[:, DS:D, :], AX.X, ALU.add)
            sb = scp.tile([128, D], F32, tag="sb")
            nc.gpsimd.dma_start(sb[:], sbD[g])
            nc.vector.tensor_tensor(sc[:], sc[:], sb[:], ALU.add)
            at = scp.tile([128, D], F32, tag="at")
            sm = scp.tile([128, 1], F32, tag="sm")
            nc.scalar.activation(at[:], sc[:], ACTF.Exp, accum_out=sm[:])
            rc = scp.tile([128, 1], F32, tag="rc")
            nc.vector.reciprocal(rc[:], sm[:])
            nc.vector.tensor_scalar(at[:], at[:], rc[:], None, ALU.mult)

            # next group's phase 2 keeps PE/ACT busy while DVE does
            # this group's softmax + weighted sum
            gva_next = None
            if g + 1 < G:
                gva_next = phase2(phase1(g + 1))

            # ---- phase 4: V = sum_d attn_d * gv_d (DVE) ----
            V = vp.tile([128, H], F32, tag="V")
            nc.vector.tensor_scalar(V[:], gva[:, 0, 0:H], at[:, 0:1], None, ALU.mult)
            for d in range(1, D):
                nc.vector.scalar_tensor_tensor(
                    V[:], gva[:, d, 0:H], at[:, d:d + 1], V[:], ALU.mult, ALU.add)

            # ---- phase 5: out = V @ w_v2 + b_v2 ----
            vt_ps = ps8.tile([128, 4, CHUNK], F32, tag="ps")
            for k in range(2):
                nc.tensor.transpose(vt_ps[:, k, 0:128], V[:, bass.ts(k, 128)], idn)
            vt = vp.tile([128, 2, 128], BF16, tag="vt")
            for k in range(2):
                nc.vector.tensor_copy(vt[:, k, :], vt_ps[:, k, 0:128])
            fo = ps8.tile([128, 4, CHUNK], F32, tag="ps")
            if not zero_bv2:
                nc.tensor.matmul(fo[:, 0, 0:H], r(ones1), r(C[0:1, O_BV:O_BV + H]),
                                 start=True, stop=False)
            for k in range(2):
                nc.tensor.matmul(fo[:, 0, 0:H], r(vt[:, k, :]),
                                 r(C[:, O_WV + k * H:O_WV + (k + 1) * H]),
                                 start=(zero_bv2 and k == 0), stop=(k == 1))
            ot = outp.tile([128, H], F32, tag="ot")
            nc.scalar.copy(ot[:], fo[:, 0, 0:H])
            nc.sync.dma_start(outD[bass.ts(g, 128)], ot[:])
            gva = gva_next

    nc.compile()
    return nc


def _prep(inputs):
    import ml_dtypes
    BF = ml_dtypes.bfloat16

    a1 = np.asarray(inputs["atom1_idx"]).reshape(B * W, D)
    a2 = np.asarray(inputs["atom2_idx"]).reshape(B * W, D)
    dist = np.asarray(inputs["distances"], dtype=np.float32).reshape(B * W, D)
    mask = np.asarray(inputs["mask"]).astype(np.float32).reshape(B * W, D)
    dm = dist * mask
    sbias = (mask - 1.0) * 1e4

    ae = np.asarray(inputs["atom_embed"], dtype=np.float32).copy()
    ae[NT - 1] = 0.0
    w_in = np.asarray(inputs["w_in"], dtype=np.float32)

    win = np.zeros((CIN, H), np.float32)
    win[0:2 * E] = w_in[0:2 * E]
    win[2 * E] = w_in[2 * E]
    win[2 * E + 1] = np.asarray(inputs["b_in"], dtype=np.float32)
    win16 = win.astype(BF)

    consts = np.zeros((128, CR), np.float32)
    w_v1 = np.asarray(inputs["w_v1"], dtype=np.float32)
    w_a1 = np.asarray(inputs["w_a1"], dtype=np.float32)
    wcat = np.concatenate([w_v1, w_a1], axis=1)          # [256, 320]
    consts[:, O_WC:O_WC + 320] = wcat[0:128]
    consts[:, O_WC + 320:O_WC + 640] = wcat[128:256]
    wv2 = np.asarray(inputs["w_v2"], dtype=np.float32)
    consts[:, O_WV:O_WV + H] = wv2[0:128]
    consts[:, O_WV + H:O_WV + 2 * H] = wv2[128:256]
    consts[0, O_ON:O_ON + 128] = 1.0
    consts[0, O_BC:O_BC + 320] = np.concatenate(
        [np.asarray(inputs["b_v1"], dtype=np.float32),
         np.asarray(inputs["b_a1"], dtype=np.float32)])
    consts[0, O_BV:O_BV + H] = np.asarray(inputs["b_v2"], dtype=np.float32)
    constsf = np.zeros((128, CF), np.float32)
    constsf[:, O_ID:O_ID + 128] = np.eye(128, dtype=np.float32)
    wa2 = np.asarray(inputs["w_a2"], dtype=np.float32)[:, 0]
    constsf[:, O_WA:O_WA + 128] = np.tile(wa2, 2)[None, :]

    e1 = ae[a1]                        # [B*W, D, E]
    e2 = ae[a2]

    maps = []
    for c in range(NCORES):
        s = slice(c * PC, (c + 1) * PC)
        m = dict(constsr=consts.astype(BF), constsf=constsf, win=win16)
        comb = np.empty((G, CIN, 128 * D), np.float32)
        comb[:, 0:E] = e1[s].reshape(G, 128, D, E).transpose(0, 3, 2, 1).reshape(
            G, E, 128 * D)
        comb[:, E:2 * E] = e2[s].reshape(G, 128, D, E).transpose(0, 3, 2, 1).reshape(
            G, E, 128 * D)
        comb[:, 2 * E] = dm[s].reshape(G, 128, D).transpose(0, 2, 1).reshape(
            G, 128 * D)
        comb[:, 2 * E + 1] = 1.0
        m["comb"] = comb.astype(BF)
        m["sbias"] = sbias[s].reshape(G, 128, D).astype(np.float32)
        maps.append(m)
    return maps, mask


def kernel(**inputs):
    global LAST_EXEC_NS
    maps, mask = _prep(inputs)
    zb1 = (not np.any(np.asarray(inputs["b_v1"]))) and (
        not np.any(np.asarray(inputs["b_a1"])))
    zb2 = not np.any(np.asarray(inputs["b_v2"]))
    nc = build_nc(None, zero_bcat=zb1, zero_bv2=zb2)
    res = run_bass_kernel_spmd(nc, maps, list(range(NCORES)), trace=TRACE)
    LAST_EXEC_NS = res.exec_time_ns
    out = np.concatenate([res.results[c]["out"] for c in range(NCORES)], axis=0)
    out = out.reshape(B, W, H)
    any_valid = mask.reshape(B, W, D).any(axis=2)
    fb = np.asarray(inputs["fallback"], dtype=np.float32)
    out = np.where(any_valid[..., None], out, fb[None, None, :])
    return out.astype(np.float32)


if __name__ == "__main__":
    nc = build_nc()
    print("build ok")


# revision 20
# speedup vs baseline: 1.0256x; 1.0156x over previous
"""DistanceAttentionPerPosition Trainium2 kernel (8-core data parallel).

Math restructure vs the reference:
  hidden = gelu([e1; e2; d*mask; 1] @ [w_in; b_in])   (embeddings gathered on
  host into a [66, edges] bf16 operand, d-major edge order; contraction 66)
  scores = gelu(hidden@w_a1)@w_a2  (b_a2 cancels in softmax)
  out = (sum_d attn_d * gelu(hidden@w_v1 + b_v1)) @ w_v2 + b_v2   (sum attn = 1)
Per core: 512 positions x 64 edges; 4 groups of 128 positions.

All matmul operands are bf16 (f32 PSUM accumulate). The bias matmuls are
skipped when the host sees all-zero b_v1/b_a1/b_v2 (build-time constant
folding; nonzero biases take the general path). Per group, phase 2 of the
NEXT group is emitted between softmax and the weighted sum so the PE/ACT
engines stay busy while the DVE drains the attention reduction; PSUM is one
unified 2x[128,4,512] pool so gelu ops cover 4 slots each.

Built on bacc.Bacc (its generate_event_semaphores pass splits multi-waits that
this walrus rejects).
"""

import sys
import numpy as np

sys.path.insert(0, "/opt/trn_rl_repo")

from contextlib import ExitStack

import concourse.bass as bass
import concourse.bacc as bacc
import concourse.tile as tile
from concourse import mybir
from concourse.bass_utils import run_bass_kernel_spmd

F32 = mybir.dt.float32
AX = mybir.AxisListType
ALU = mybir.AluOpType
ACTF = mybir.ActivationFunctionType

B, W, D = 16, 256, 64
E, H = 32, 256
NT = 101
NCORES = 8
PC = (B * W) // NCORES      # positions per core = 512
NE = PC * D                 # edges per core = 32768
G = PC // 128               # groups per core = 4
CHUNK = 512
NCHUNK = NE // CHUNK        # 64
NJ = D // 2
CIN = 2 * E + 2             # phase-1 contraction rows: e1, e2, d*mask, ones

# bf16 const pack (matmul operands), one [128, CR] tensor
O_WC = 0            # wcat  [128, 2*320]
O_WV = 640          # wv2   [128, 2*256]
O_ON = 1152         # ones  [row0, 128]
O_BC = 1280         # bcat  [row0, 320]
O_BV = 1600         # bv2   [row0, 256]
CR = 1856
# f32 const pack, one [128, CF] tensor
O_ID = 0            # ident [128, 128]
O_WA = 128          # wa2b  [128, 2*64]
CF = 256

TRACE = False
LAST_EXEC_NS = None


def build_nc(gelu=None, zero_bcat=False, zero_bv2=False):
    gelu = ACTF.Gelu if gelu is None else gelu
    ACTF_Gelu = gelu
    nc = bacc.Bacc(None, target_bir_lowering=False)

    F32R = mybir.dt.float32r
    BF16 = mybir.dt.bfloat16
    combD = nc.declare_dram_parameter("comb", [G, CIN, 128 * D], BF16, isOutput=False)
    winD = nc.declare_dram_parameter("win", [CIN, H], BF16, isOutput=False)
    sbD = nc.declare_dram_parameter("sbias", [G, 128, D], F32, isOutput=False)
    cD = nc.declare_dram_parameter("constsr", [128, CR], BF16, isOutput=False)
    cfD = nc.declare_dram_parameter("constsf", [128, CF], F32, isOutput=False)
    outD = nc.declare_dram_parameter("out", [PC, H], F32, isOutput=True)

    with tile.TileContext(nc) as tc, ExitStack() as ctx:
        const = ctx.enter_context(tc.tile_pool(name="const", bufs=1))
        cbp = ctx.enter_context(tc.tile_pool(name="cbp", bufs=2))
        gpp = ctx.enter_context(tc.tile_pool(name="gpp", bufs=2))
        gvp = ctx.enter_context(tc.tile_pool(name="gvp", bufs=2))
        scp = ctx.enter_context(tc.tile_pool(name="scp", bufs=2))
        vp = ctx.enter_context(tc.tile_pool(name="vp", bufs=2))
        outp = ctx.enter_context(tc.tile_pool(name="outp", bufs=2))
        scsp = ctx.enter_context(tc.tile_pool(name="scsp", bufs=1))
        ps8 = ctx.enter_context(
            tc.tile_pool(name="ps8", bufs=2, space=bass.MemorySpace.PSUM))

        C = const.tile([128, CR], BF16, tag="constsr")
        nc.sync.dma_start(C[:], cD[:])
        Cf = const.tile([128, CF], F32, tag="constsf")
        nc.sync.dma_start(Cf[:], cfD[:])
        Wb = const.tile([CIN, H], BF16, tag="win")
        nc.sync.dma_start(Wb[:], winD[:])
        def r(ap):
            return ap
        ones1 = C[0:1, O_ON:O_ON + 128]
        idn = Cf[:, O_ID:O_ID + 128]

        def phase1(g):
            gp = gpp.tile([128, 2, 128 * D], BF16, tag="gp")
            cb = cbp.tile([CIN, 128 * D], BF16, tag="cb")
            nc.sync.dma_start(cb[:], combD[g])
            for cp in range(NCHUNK // G // 2):
                pp = ps8.tile([128, 4, CHUNK], F32, tag="ps")
                for cc in range(2):
                    c = 2 * cp + cc
                    for m in range(2):
                        nc.tensor.matmul(pp[:, 2 * cc + m, :],
                                         Wb[:, m * 128:(m + 1) * 128],
                                         cb[:, c * CHUNK:(c + 1) * CHUNK],
                                         start=True, stop=True)
                nc.scalar.activation(
                    gp[:, :, cp * 2 * CHUNK:(cp + 1) * 2 * CHUNK].rearrange(
                        "p m (cc e) -> p cc m e", cc=2),
                    pp[:, :, :], ACTF_Gelu)
            return gp

        def phase1_tile(state, cp):
            gp, cb = state
            pp = ps8.tile([128, 4, CHUNK], F32, tag="ps")
            for cc in range(2):
                c = 2 * cp + cc
                for m in range(2):
                    nc.tensor.matmul(pp[:, 2 * cc + m, :],
                                     Wb[:, m * 128:(m + 1) * 128],
                                     cb[:, c * CHUNK:(c + 1) * CHUNK],
                                     start=True, stop=True)
            nc.scalar.activation(
                gp[:, :, cp * 2 * CHUNK:(cp + 1) * 2 * CHUNK].rearrange(
                    "p m (cc e) -> p cc m e", cc=2),
                pp[:, :, :], ACTF_Gelu)

        def phase2(gp):
            # values + attention-logit inputs: gva = gelu(hidden @ wcat [+ bcat])
            gva = gvp.tile([128, D, 320], BF16, tag="gva")
            for j in range(D // 4):
                vps = ps8.tile([128, 4, CHUNK], F32, tag="ps")
                for dd in range(4):
                    d = 4 * j + dd
                    if not zero_bcat:
                        nc.tensor.matmul(vps[:, dd, 0:320], r(ones1),
                                         r(C[0:1, O_BC:O_BC + 320]),
                                         start=True, stop=False)
                    for k in range(2):
                        nc.tensor.matmul(
                            vps[:, dd, 0:320],
                            r(gp[:, k, d * 128:(d + 1) * 128]),
                            r(C[:, O_WC + k * 320:O_WC + (k + 1) * 320]),
                            start=(zero_bcat and k == 0), stop=(k == 1))
                nc.scalar.activation(gva[:, 4 * j:4 * j + 4, :], vps[:, 0:4, 0:320],
                                     ACTF_Gelu)
            return gva

        def phase1(g):
            gp = gpp.tile([128, 2, 128 * D], BF16, tag="gp")
            cb = cbp.tile([CIN, 128 * D], BF16, tag="cb")
            nc.sync.dma_start(cb[:], combD[g])
            for cp in range(NCHUNK // G // 2):
                phase1_tile((gp, cb), cp)
            return gp

        gva = phase2(phase1(0))
        for g in range(G):
            # ---- phase 3: scores + softmax over d (DVE/ACT) ----
            sc = scp.tile([128, D], F32, tag="sc")
            scs = scsp.tile([128, D, 64], F32, tag="scs")
            DS = 48
            nc.vector.tensor_tensor(
                scs[:, 0:DS, :], gva[:, 0:DS, 256:320],
                Cf[:, O_WA:O_WA + 64][:, None, :].broadcast_to([128, DS, 64]),
                ALU.mult)
            nc.vector.tensor_reduce(sc[:, 0:DS], scs[:, 0:DS, :], AX.X, ALU.add)
            nc.vector.tensor_tensor(
                scs[:, DS:D, :], gva[:, DS:D, 256:320],
                Cf[:, O_WA:O_WA + 64][:, None, :].broadcast_to([128, D - DS, 64]),
                ALU.mult)
            nc.vector.tensor_reduce(sc[:, DS:D], scs[:, DS:D, :], AX.X, ALU.add)
            sb = scp.tile([128, D], F32, tag="sb")
            nc.gpsimd.dma_start(sb[:], sbD[g])
            nc.vector.tensor_tensor(sc[:], sc[:], sb[:], ALU.add)
            at = scp.tile([128, D], F32, tag="at")
            sm = scp.tile([128, 1], F32, tag="sm")
            nc.scalar.activation(at[:], sc[:], ACTF.Exp, accum_out=sm[:])
            rc = scp.tile([128, 1], F32, tag="rc")
            nc.vector.reciprocal(rc[:], sm[:])

            # next group's phase 2 keeps PE/ACT busy while DVE does
            # this group's softmax + weighted sum
            gva_next = None
            if g + 1 < G:
                gva_next = phase2(phase1(g + 1))

            # ---- phase 4: V = sum_d attn_d * gv_d (DVE) ----
            V = vp.tile([128, H], F32, tag="V")
            nc.vector.tensor_scalar(V[:], gva[:, 0, 0:H], at[:, 0:1], None, ALU.mult)
            for d in range(1, D):
                nc.vector.scalar_tensor_tensor(
                    V[:], gva[:, d, 0:H], at[:, d:d + 1], V[:], ALU.mult, ALU.add)

            # ---- phase 5: out = V @ w_v2 + b_v2 ----
            vt_ps = ps8.tile([128, 4, CHUNK], F32, tag="ps")
            for k in range(2):
                nc.tensor.transpose(vt_ps[:, k, 0:128], V[:, bass.ts(k, 128)], idn)
            vt = vp.tile([128, 2, 128], BF16, tag="vt")
            for k in range(2):
                nc.vector.tensor_copy(vt[:, k, :], vt_ps[:, k, 0:128])
            fo = ps8.tile([128, 4, CHUNK], F32, tag="ps")
            if not zero_bv2:
                nc.tensor.matmul(fo[:, 0, 0:H], r(ones1), r(C[0:1, O_BV:O_BV + H]),
                                 start=True, stop=False)
            for k in range(2):
                nc.tensor.matmul(fo[:, 0, 0:H], r(vt[:, k, :]),
                                 r(C[:, O_WV + k * H:O_WV + (k + 1) * H]),
                                 start=(zero_bv2 and k == 0), stop=(k == 1))
            ot = outp.tile([128, H], F32, tag="ot")
            nc.scalar.mul(ot[:], fo[:, 0, 0:H], rc[:])
            nc.sync.dma_start(outD[bass.ts(g, 128)], ot[:])
            gva = gva_next

    nc.compile()
    return nc


def _prep(inputs):
    import ml_dtypes
    BF = ml_dtypes.bfloat16

    a1 = np.asarray(inputs["atom1_idx"]).reshape(B * W, D)
    a2 = np.asarray(inputs["atom2_idx"]).reshape(B * W, D)
    dist = np.asarray(inputs["distances"], dtype=np.float32).reshape(B * W, D)
    mask = np.asarray(inputs["mask"]).astype(np.float32).reshape(B * W, D)
    dm = dist * mask
    sbias = (mask - 1.0) * 1e4

    ae = np.asarray(inputs["atom_embed"], dtype=np.float32).copy()
    ae[NT - 1] = 0.0
    w_in = np.asarray(inputs["w_in"], dtype=np.float32)

    win = np.zeros((CIN, H), np.float32)
    win[0:2 * E] = w_in[0:2 * E]
    win[2 * E] = w_in[2 * E]
    win[2 * E + 1] = np.asarray(inputs["b_in"], dtype=np.float32)
    win16 = win.astype(BF)

    consts = np.zeros((128, CR), np.float32)
    w_v1 = np.asarray(inputs["w_v1"], dtype=np.float32)
    w_a1 = np.asarray(inputs["w_a1"], dtype=np.float32)
    wcat = np.concatenate([w_v1, w_a1], axis=1)          # [256, 320]
    consts[:, O_WC:O_WC + 320] = wcat[0:128]
    consts[:, O_WC + 320:O_WC + 640] = wcat[128:256]
    wv2 = np.asarray(inputs["w_v2"], dtype=np.float32)
    consts[:, O_WV:O_WV + H] = wv2[0:128]
    consts[:, O_WV + H:O_WV + 2 * H] = wv2[128:256]
    consts[0, O_ON:O_ON + 128] = 1.0
    consts[0, O_BC:O_BC + 320] = np.concatenate(
        [np.asarray(inputs["b_v1"], dtype=np.float32),
         np.asarray(inputs["b_a1"], dtype=np.float32)])
    consts[0, O_BV:O_BV + H] = np.asarray(inputs["b_v2"], dtype=np.float32)
    constsf = np.zeros((128, CF), np.float32)
    constsf[:, O_ID:O_ID + 128] = np.eye(128, dtype=np.float32)
    wa2 = np.asarray(inputs["w_a2"], dtype=np.float32)[:, 0]
    constsf[:, O_WA:O_WA + 128] = np.tile(wa2, 2)[None, :]

    e1 = ae[a1]                        # [B*W, D, E]
    e2 = ae[a2]

    maps = []
    for c in range(NCORES):
        s = slice(c * PC, (c + 1) * PC)
        m = dict(constsr=consts.astype(BF), constsf=constsf, win=win16)
        comb = np.empty((G, CIN, 128 * D), np.float32)
        comb[:, 0:E] = e1[s].reshape(G, 128, D, E).transpose(0, 3, 2, 1).reshape(
            G, E, 128 * D)
        comb[:, E:2 * E] = e2[s].reshape(G, 128, D, E).transpose(0, 3, 2, 1).reshape(
            G, E, 128 * D)
        comb[:, 2 * E] = dm[s].reshape(G, 128, D).transpose(0, 2, 1).reshape(
            G, 128 * D)
        comb[:, 2 * E + 1] = 1.0
        m["comb"] = comb.astype(BF)
        m["sbias"] = sbias[s].reshape(G, 128, D).astype(np.float32)
        maps.append(m)
    return maps, mask


def kernel(**inputs):
    global LAST_EXEC_NS
    maps, mask = _prep(inputs)
    zb1 = (not np.any(np.asarray(inputs["b_v1"]))) and (
        not np.any(np.asarray(inputs["b_a1"])))
    zb2 = not np.any(np.asarray(inputs["b_v2"]))
    nc = build_nc(None, zero_bcat=zb1, zero_bv2=zb2)
    res = run_bass_kernel_spmd(nc, maps, list(range(NCORES)), trace=TRACE)
    LAST_EXEC_NS = res.exec_time_ns
    out = np.concatenate([res.results[c]["out"] for c in range(NCORES)], axis=0)
    out = out.reshape(B, W, H)
    any_valid = mask.reshape(B, W, D).any(axis=2)
    fb = np.asarray(inputs["fallback"], dtype=np.float32)
    out = np.where(any_valid[..., None], out, fb[None, None, :])
    return out.astype(np.float32)


if __name__ == "__main__":
    nc = build_nc()
    print("build ok")
